# revision 12
# baseline (speedup 1.0000x reference)
"""Adstock transform on 8 trn2 cores — J=8 polyphase decimation, DVE+ScalarE.

r[b, t, c] = x[b, t, c] + d[c] * r[b, t-1, c],  d = sigmoid(decay)

The DVE scan op runs at ~2 cyc/elem (feedback-limited), so a direct scan costs
~137us/core.  Instead, de-interleave time into 8 phases (host-side permute):
  t = 8k + p,  phase arrays of length K = T/8 = 1024 per batch.
Build the 8-step block sums z8[k] = sum_{j<8} d^j x[8k+7-j] with a tree of
(scale, add) passes, scan only z8 (T/8 elements, decay d^8) -> R[k] = r[8k+7],
then reconstruct phases 0-6 with one (scale, add) each:
  r_ph = partial + d^j * carrier   (carrier = R[k-1] or an earlier phase).

Engine split: per-partition scales on ScalarE (activation Copy, ~0.85 ns/elem)
and DVE tensor_scalar (4x mode) for the latency-critical chain scales; adds on
DVE tensor_tensor (2x mode).  GpSimd is intentionally NOT used: its SBUF
traffic was measured to slow concurrent DVE ops ~2.4x (contention), costing
more than it offloads.  Batches are processed in fused pairs so elementwise
ops run at FD=2048 and DMAs move 4 MiB slabs.

Layout: host permutes x to phase-major c-rows [4, C, 16384] bf16 per core
(x[i, c, p*2048 + j*1024 + k] = x_orig[2i+j, 8k+p, c]); bf16 I/O halves HBM
traffic (measured end-to-end rel err ~1e-2 vs the 2e-2 gate).
"""

import numpy as np
import ml_dtypes

import concourse.bacc as bacc
import concourse.mybir as mybir
from concourse.bass_utils import run_bass_kernel_spmd
from concourse.tile import TileContext

F32 = mybir.dt.float32
BF16 = mybir.dt.bfloat16
_BF16_NP = ml_dtypes.bfloat16

B, T, C = 64, 8192, 128
NCORES = 8
B_LOC = B // NCORES  # 8 batches per core
J = 8                # decimation factor (phases)
K = T // J           # 1024 scan steps per phase per batch
P = 2                # batches fused per pair
NP = B_LOC // P      # 4 pairs per core
F = P * K            # 2048: fused elementwise op width
TP = P * T           # 16384: free size of one pair slab


def build_nc():
    nc = bacc.Bacc("TRN2", target_bir_lowering=False, debug=False)
    x = nc.dram_tensor("x", [NP, C, TP], BF16, kind="ExternalInput").ap()
    dpow = nc.dram_tensor("dpow", [C, 4], F32, kind="ExternalInput").ap()
    y = nc.dram_tensor("y", [NP, C, TP], BF16, kind="ExternalOutput").ap()

    M = mybir.AluOpType
    AF = mybir.ActivationFunctionType

    with TileContext(nc) as tc:
        with (
            tc.tile_pool(name="const", bufs=1) as cpool,
            tc.tile_pool(name="inp", bufs=2) as inp,
            tc.tile_pool(name="outp", bufs=10) as outp,
            tc.tile_pool(name="rp", bufs=3) as rp,
            tc.tile_pool(name="sp", bufs=2) as sp,
            tc.tile_pool(name="tp", bufs=2) as tp,
        ):
            dp = cpool.tile([C, 4], F32)
            nc.sync.dma_start(out=dp, in_=dpow)
            d1 = dp[:, 0:1]
            d2 = dp[:, 1:2]
            d4 = dp[:, 2:3]
            d8 = dp[:, 3:4]
            d8_bc = d8.broadcast_to([C, K])

            for i in range(NP):
                # split loads: 4x 1MiB (compute starts sooner, finer overlap)
                ld = inp.tile([C, TP], BF16, tag="in")
                for q in range(4):
                    nc.sync.dma_start(
                        out=ld[:, q * 2 * F : (q + 1) * 2 * F],
                        in_=x[i, :, q * 2 * F : (q + 1) * 2 * F],
                    )
                # phase slice (both batches of the pair): FD=2048
                xp = [ld[:, p * F : (p + 1) * F] for p in range(J)]
                # phase slice of one batch j: FD=1024
                xpj = [
                    [ld[:, p * F + j * K : p * F + (j + 1) * K] for j in range(P)]
                    for p in range(J)
                ]

                rt = rp.tile([C, 2 * K + 2], BF16, tag="r")

                def fma(tag, dcol, src, addend, fd=F):
                    """tile = dcol*src (ScalarE) ; tile += addend (DVE)."""
                    t = sp.tile([C, fd], BF16, tag=tag)
                    nc.scalar.activation(out=t, in_=src, func=AF.Copy, scale=dcol)
                    nc.vector.tensor_tensor(out=t, in0=t, in1=addend, op=M.add)
                    return t

                # ---- prep tree: z8[k] = sum_{j<8} d^j x[8k+7-j] ----
                s1_01 = fma("s1_01", d1, xp[0], xp[1])
                s1_23 = fma("s1_23", d1, xp[2], xp[3])
                s1_45 = fma("s1_45", d1, xp[4], xp[5])
                s1_67 = fma("s1_67", d1, xp[6], xp[7])
                s2_03 = fma("s2_03", d2, s1_01, s1_23)
                s2_47 = fma("s2_47", d2, s1_45, s1_67)
                z8 = fma("z8", d4, s2_03, s2_47)

                # ---- per-batch block scans: R[k] = d^8 R[k-1] + z8[k] ----
                # rt columns: [0]=0-pad | [1..K]=R_j0 | [K+1]=0-pad | [K+2..2K+1]=R_j1
                nc.vector.memset(rt[:, 0:1], 0.0)
                nc.vector.memset(rt[:, K + 1 : K + 2], 0.0)
                S = [rt[:, 0:K], rt[:, K + 1 : 2 * K + 1]]
                R7 = [rt[:, 1 : K + 1], rt[:, K + 2 : 2 * K + 2]]
                for j in range(P):
                    nc.vector.tensor_tensor_scan(
                        out=R7[j],
                        data0=d8_bc,
                        data1=z8[:, j * K : (j + 1) * K],
                        initial=0.0,
                        op0=M.mult,
                        op1=M.add,
                    )
                # phase-7 stores issued immediately so rt recycles early.
                # All stores ride the (otherwise idle) GpSimd SWDGE queue so
                # they never FIFO-block the next pair's ScalarE/load work.
                nc.gpsimd.dma_start(out=y[i, :, 7 * F : 7 * F + K], in_=R7[0])
                nc.gpsimd.dma_start(out=y[i, :, 7 * F + K : TP], in_=R7[1])

                # ---- reconstruction into per-phase tiles, eager stores ----
                ph_t = {
                    p: outp.tile([C, F], BF16, tag="pho", name=f"pho_{i}_{p}")
                    for p in range(7)
                }

                def store(p):
                    nc.gpsimd.dma_start(
                        out=y[i, :, p * F : (p + 1) * F], in_=ph_t[p]
                    )

                # S-based phases, per batch j (FD=1024); scales on DVE TS so
                # they don't sit behind scan-dependent ops in the ScalarE FIFO
                for j in range(P):
                    js = slice(j * K, (j + 1) * K)
                    a0 = tp.tile([C, K], BF16, tag="a0")
                    nc.vector.tensor_scalar(
                        out=a0, in0=S[j], scalar1=d1, scalar2=None, op0=M.mult
                    )
                    nc.vector.tensor_tensor(
                        out=ph_t[0][:, js], in0=a0, in1=xpj[0][j], op=M.add
                    )
                    a1 = tp.tile([C, K], BF16, tag="a1")
                    nc.vector.tensor_scalar(
                        out=a1, in0=S[j], scalar1=d2, scalar2=None, op0=M.mult
                    )
                    nc.vector.tensor_tensor(
                        out=ph_t[1][:, js], in0=a1, in1=s1_01[:, js], op=M.add
                    )
                    a3 = tp.tile([C, K], BF16, tag="a3")
                    nc.vector.tensor_scalar(
                        out=a3, in0=S[j], scalar1=d4, scalar2=None, op0=M.mult
                    )
                    nc.vector.tensor_tensor(
                        out=ph_t[3][:, js], in0=a3, in1=s2_03[:, js], op=M.add
                    )
                store(0)
                store(1)
                store(3)

                # chained phases (FD=2048), chain scales on ScalarE
                def chain(tag, dcol, src, addend, p_dst):
                    t = tp.tile([C, F], BF16, tag=tag)
                    nc.scalar.activation(out=t, in_=src, func=AF.Copy, scale=dcol)
                    nc.vector.tensor_tensor(
                        out=ph_t[p_dst], in0=t, in1=addend, op=M.add
                    )
                    store(p_dst)

                chain("ch0", d1, ph_t[1], xp[2], 2)
                chain("ch1", d1, ph_t[3], xp[4], 4)
                chain("ch0", d2, ph_t[3], s1_45, 5)
                chain("ch1", d1, ph_t[5], xp[6], 6)
    nc.finalize()
    return nc


_NC_CACHE = {}


def _get_nc():
    if "nc" not in _NC_CACHE:
        _NC_CACHE["nc"] = build_nc()
    return _NC_CACHE["nc"]


def _make_dpow(decay: np.ndarray) -> np.ndarray:
    d = 1.0 / (1.0 + np.exp(-decay.astype(np.float64)))  # [C]
    dp = np.stack([d, d**2, d**4, d**8], axis=1)  # [C, 4]
    return dp.astype(np.float32).copy()


def _permute_in(xc: np.ndarray) -> np.ndarray:
    """[b_loc, T, C] f32 -> pair-fused phase-major [NP, C, TP] bf16."""
    xp = xc.reshape(NP, P, K, J, C).transpose(0, 4, 3, 1, 2)  # [i, c, p, j, k]
    return np.ascontiguousarray(xp).reshape(NP, C, TP).astype(_BF16_NP)


def _unpermute_out(yp: np.ndarray) -> np.ndarray:
    """pair-fused phase-major [NP, C, TP] bf16 -> [b_loc, T, C] f32."""
    ya = np.asarray(yp).astype(np.float32).reshape(NP, C, J, P, K)
    return np.ascontiguousarray(ya.transpose(0, 3, 4, 2, 1)).reshape(B_LOC, T, C)


def make_in_maps(x, decay):
    x = np.asarray(x, dtype=np.float32)
    dp = _make_dpow(np.asarray(decay))
    return [
        {"x": _permute_in(x[i * B_LOC : (i + 1) * B_LOC]), "dpow": dp}
        for i in range(NCORES)
    ]


def run(x, decay, trace=False, tmpdir=None, trace_cores=None):
    nc = _get_nc()
    in_maps = make_in_maps(x, decay)
    res = run_bass_kernel_spmd(
        nc,
        in_maps,
        list(range(NCORES)),
        trace=trace,
        tmpdir=tmpdir,
        trace_cores=trace_cores,
    )
    out = np.concatenate([_unpermute_out(r["y"]) for r in res.results], axis=0)
    return out, res


def kernel(x: np.ndarray, decay: np.ndarray) -> np.ndarray:
    out, _ = run(x, decay)
    return out


# revision 13
# speedup vs baseline: 1.0155x; 1.0155x over previous
"""Adstock transform on 8 trn2 cores — J=8 polyphase decimation, DVE+ScalarE.

r[b, t, c] = x[b, t, c] + d[c] * r[b, t-1, c],  d = sigmoid(decay)

The DVE scan op runs at ~2 cyc/elem (feedback-limited), so a direct scan costs
~137us/core.  Instead, de-interleave time into 8 phases (host-side permute):
  t = 8k + p,  phase arrays of length K = T/8 = 1024 per batch.
Build the 8-step block sums z8[k] = sum_{j<8} d^j x[8k+7-j] with a tree of
(scale, add) passes, scan only z8 (T/8 elements, decay d^8) -> R[k] = r[8k+7],
then reconstruct phases 0-6 with one (scale, add) each:
  r_ph = partial + d^j * carrier   (carrier = R[k-1] or an earlier phase).

Engine split: per-partition scales on ScalarE (activation Copy, ~0.85 ns/elem)
and DVE tensor_scalar (4x mode) for the latency-critical chain scales; adds on
DVE tensor_tensor (2x mode).  GpSimd is intentionally NOT used: its SBUF
traffic was measured to slow concurrent DVE ops ~2.4x (contention), costing
more than it offloads.  Batches are processed in fused pairs so elementwise
ops run at FD=2048 and DMAs move 4 MiB slabs.

Layout: host permutes x to phase-major c-rows [4, C, 16384] bf16 per core
(x[i, c, p*2048 + j*1024 + k] = x_orig[2i+j, 8k+p, c]); bf16 I/O halves HBM
traffic (measured end-to-end rel err ~1e-2 vs the 2e-2 gate).
"""

import numpy as np
import ml_dtypes

import concourse.bacc as bacc
import concourse.mybir as mybir
from concourse.bass_utils import run_bass_kernel_spmd
from concourse.tile import TileContext

F32 = mybir.dt.float32
BF16 = mybir.dt.bfloat16
_BF16_NP = ml_dtypes.bfloat16

B, T, C = 64, 8192, 128
NCORES = 8
B_LOC = B // NCORES  # 8 batches per core
J = 8                # decimation factor (phases)
K = T // J           # 1024 scan steps per phase per batch
P = 2                # batches fused per pair
NP = B_LOC // P      # 4 pairs per core
F = P * K            # 2048: fused elementwise op width
TP = P * T           # 16384: free size of one pair slab


def build_nc():
    nc = bacc.Bacc("TRN2", target_bir_lowering=False, debug=False)
    x = nc.dram_tensor("x", [NP, C, TP], BF16, kind="ExternalInput").ap()
    dpow = nc.dram_tensor("dpow", [C, 4], F32, kind="ExternalInput").ap()
    y = nc.dram_tensor("y", [NP, C, TP], BF16, kind="ExternalOutput").ap()

    M = mybir.AluOpType
    AF = mybir.ActivationFunctionType

    with TileContext(nc) as tc:
        with (
            tc.tile_pool(name="const", bufs=1) as cpool,
            tc.tile_pool(name="inp", bufs=2) as inp,
            tc.tile_pool(name="outp", bufs=10) as outp,
            tc.tile_pool(name="rp", bufs=3) as rp,
            tc.tile_pool(name="sp", bufs=2) as sp,
            tc.tile_pool(name="tp", bufs=2) as tp,
        ):
            dp = cpool.tile([C, 4], F32)
            nc.sync.dma_start(out=dp, in_=dpow)
            d1 = dp[:, 0:1]
            d2 = dp[:, 1:2]
            d4 = dp[:, 2:3]
            d8 = dp[:, 3:4]
            d8_bc = d8.broadcast_to([C, K])

            for i in range(NP):
                # split loads: 4x 1MiB (compute starts sooner, finer overlap)
                ld = inp.tile([C, TP], BF16, tag="in")
                for q in range(4):
                    nc.sync.dma_start(
                        out=ld[:, q * 2 * F : (q + 1) * 2 * F],
                        in_=x[i, :, q * 2 * F : (q + 1) * 2 * F],
                    )
                # phase slice (both batches of the pair): FD=2048
                xp = [ld[:, p * F : (p + 1) * F] for p in range(J)]
                # phase slice of one batch j: FD=1024
                xpj = [
                    [ld[:, p * F + j * K : p * F + (j + 1) * K] for j in range(P)]
                    for p in range(J)
                ]

                rt = rp.tile([C, 2 * K + 2], BF16, tag="r")

                def fma(tag, dcol, src, addend, fd=F):
                    """tile = dcol*src (ScalarE) ; tile += addend (DVE)."""
                    t = sp.tile([C, fd], BF16, tag=tag)
                    nc.scalar.activation(out=t, in_=src, func=AF.Copy, scale=dcol)
                    nc.vector.tensor_tensor(out=t, in0=t, in1=addend, op=M.add)
                    return t

                # ---- prep tree: z8[k] = sum_{j<8} d^j x[8k+7-j] ----
                s1_01 = fma("s1_01", d1, xp[0], xp[1])
                s1_23 = fma("s1_23", d1, xp[2], xp[3])
                s1_45 = fma("s1_45", d1, xp[4], xp[5])
                s1_67 = fma("s1_67", d1, xp[6], xp[7])
                s2_03 = fma("s2_03", d2, s1_01, s1_23)
                s2_47 = fma("s2_47", d2, s1_45, s1_67)
                z8 = fma("z8", d4, s2_03, s2_47)

                # ---- per-batch block scans: R[k] = d^8 R[k-1] + z8[k] ----
                # rt columns: [0]=0-pad | [1..K]=R_j0 | [K+1]=0-pad | [K+2..2K+1]=R_j1
                nc.vector.memset(rt[:, 0:1], 0.0)
                nc.vector.memset(rt[:, K + 1 : K + 2], 0.0)
                S = [rt[:, 0:K], rt[:, K + 1 : 2 * K + 1]]
                R7 = [rt[:, 1 : K + 1], rt[:, K + 2 : 2 * K + 2]]
                for j in range(P):
                    nc.vector.tensor_tensor_scan(
                        out=R7[j],
                        data0=d8_bc,
                        data1=z8[:, j * K : (j + 1) * K],
                        initial=0.0,
                        op0=M.mult,
                        op1=M.add,
                    )
                # phase-7 stores issued immediately so rt recycles early.
                # All stores ride the (otherwise idle) GpSimd SWDGE queue so
                # they never FIFO-block the next pair's ScalarE/load work.
                nc.gpsimd.dma_start(out=y[i, :, 7 * F : 7 * F + K], in_=R7[0])
                nc.gpsimd.dma_start(out=y[i, :, 7 * F + K : TP], in_=R7[1])

                # ---- reconstruction into per-phase tiles, eager stores ----
                ph_t = {
                    p: outp.tile([C, F], BF16, tag="pho", name=f"pho_{i}_{p}")
                    for p in range(7)
                }

                def store(p):
                    nc.gpsimd.dma_start(
                        out=y[i, :, p * F : (p + 1) * F], in_=ph_t[p]
                    )

                # S-based phases, per batch j (FD=1024); scales on DVE TS so
                # they don't sit behind scan-dependent ops in the ScalarE FIFO
                for j in range(P):
                    js = slice(j * K, (j + 1) * K)
                    a0 = tp.tile([C, K], BF16, tag="a0")
                    nc.vector.tensor_scalar(
                        out=a0, in0=S[j], scalar1=d1, scalar2=None, op0=M.mult
                    )
                    nc.vector.tensor_tensor(
                        out=ph_t[0][:, js], in0=a0, in1=xpj[0][j], op=M.add
                    )
                    a1 = tp.tile([C, K], BF16, tag="a1")
                    nc.vector.tensor_scalar(
                        out=a1, in0=S[j], scalar1=d2, scalar2=None, op0=M.mult
                    )
                    nc.vector.tensor_tensor(
                        out=ph_t[1][:, js], in0=a1, in1=s1_01[:, js], op=M.add
                    )
                    a3 = tp.tile([C, K], BF16, tag="a3")
                    nc.vector.tensor_scalar(
                        out=a3, in0=S[j], scalar1=d4, scalar2=None, op0=M.mult
                    )
                    nc.vector.tensor_tensor(
                        out=ph_t[3][:, js], in0=a3, in1=s2_03[:, js], op=M.add
                    )
                store(0)
                store(1)
                store(3)

                # chained phases (FD=2048), chain scales on DVE TS (4x)
                def chain(tag, dcol, src, addend, p_dst):
                    t = tp.tile([C, F], BF16, tag=tag)
                    nc.vector.tensor_scalar(
                        out=t, in0=src, scalar1=dcol, scalar2=None, op0=M.mult
                    )
                    nc.vector.tensor_tensor(
                        out=ph_t[p_dst], in0=t, in1=addend, op=M.add
                    )
                    store(p_dst)

                chain("ch0", d1, ph_t[1], xp[2], 2)
                chain("ch1", d1, ph_t[3], xp[4], 4)
                chain("ch0", d2, ph_t[3], s1_45, 5)
                chain("ch1", d1, ph_t[5], xp[6], 6)
    nc.finalize()
    return nc


_NC_CACHE = {}


def _get_nc():
    if "nc" not in _NC_CACHE:
        _NC_CACHE["nc"] = build_nc()
    return _NC_CACHE["nc"]


def _make_dpow(decay: np.ndarray) -> np.ndarray:
    d = 1.0 / (1.0 + np.exp(-decay.astype(np.float64)))  # [C]
    dp = np.stack([d, d**2, d**4, d**8], axis=1)  # [C, 4]
    return dp.astype(np.float32).copy()


def _permute_in(xc: np.ndarray) -> np.ndarray:
    """[b_loc, T, C] f32 -> pair-fused phase-major [NP, C, TP] bf16."""
    xp = xc.reshape(NP, P, K, J, C).transpose(0, 4, 3, 1, 2)  # [i, c, p, j, k]
    return np.ascontiguousarray(xp).reshape(NP, C, TP).astype(_BF16_NP)


def _unpermute_out(yp: np.ndarray) -> np.ndarray:
    """pair-fused phase-major [NP, C, TP] bf16 -> [b_loc, T, C] f32."""
    ya = np.asarray(yp).astype(np.float32).reshape(NP, C, J, P, K)
    return np.ascontiguousarray(ya.transpose(0, 3, 4, 2, 1)).reshape(B_LOC, T, C)


def make_in_maps(x, decay):
    x = np.asarray(x, dtype=np.float32)
    dp = _make_dpow(np.asarray(decay))
    return [
        {"x": _permute_in(x[i * B_LOC : (i + 1) * B_LOC]), "dpow": dp}
        for i in range(NCORES)
    ]


def run(x, decay, trace=False, tmpdir=None, trace_cores=None):
    nc = _get_nc()
    in_maps = make_in_maps(x, decay)
    res = run_bass_kernel_spmd(
        nc,
        in_maps,
        list(range(NCORES)),
        trace=trace,
        tmpdir=tmpdir,
        trace_cores=trace_cores,
    )
    out = np.concatenate([_unpermute_out(r["y"]) for r in res.results], axis=0)
    return out, res


def kernel(x: np.ndarray, decay: np.ndarray) -> np.ndarray:
    out, _ = run(x, decay)
    return out
